# revision 4
# baseline (speedup 1.0000x reference)
"""Trainium2 Bass kernel for the RNN-T JointNetwork problem.

  enc_proj = enc_out @ W_enc + b_enc          # (B,T,1,J)
  dec_proj = dec_out @ W_dec + b_dec          # (B,1,U,J)
  joint    = tanh(enc_proj + dec_proj)        # (B,T,U,J)
  out      = joint @ W_out + b_out            # (B,T,U,V)

with B=4, T=512, U=128, D=512, J=512, V=1024.

Sharding: 8 shards over (batch, T-half); core c owns b = c//2 and T rows
[t0, t0+256) with t0 = (c%2)*256.  Each core computes its full (256,128,1024)
output slab; no collectives are needed.

Per-core design (cost-model driven; PE busy ~98%, ~451 us simulated vs a
~437 us bf16-matmul floor and ~377 us HBM store floor):
  - Host pre-processing: enc/dec/W_enc/W_dec/W_out cast to bf16 on the host
    (halves every load; every matmul runs at 1 PE cycle/row) and b_enc+b_dec
    pre-summed.  End-to-end rel err of the bf16 pipeline ~3.5e-3 (vs the
    2e-2 budget).  fp8 was evaluated and rejected: even a quarter of the
    contraction in fp8-DoubleRow costs ~2.6e-2 rel err.
  - DMA transfers serialize globally with the two HWDGE queues interleaved,
    so loads are queue-assigned to reach the device in first-use order:
    enc, w_enc, dec, w_dec, b_sum, w_out[v0], w_out[v1], b_out.
  - PE p-state warmup: 28 dummy matmuls on the identity while loads are in
    flight, so the PE clock (2.4 GHz after ~3 us of continuous busy) is
    fully ramped when real work starts.  A dummy tanh likewise preloads the
    ScalarE activation table.
  - Preamble: PE-transposes of enc/dec batch all chunks of a group into one
    full-bank PSUM tile drained by a single wide DVE copy (PE never waits
    on DVE); enc_projT accumulates in the 4-deep main PSUM pool so chunk
    drains never gate the next chunk; dec_projT is built directly in [j, u]
    form with 16 narrow matmuls (full speed post-warmup, no extra
    transpose pass).
  - Main loop per t row: 4 tanh activations (per-jc bf16 tiles; ScalarE
    bias port adds enc_projT[:, t]), 8 bf16 matmuls into 2 PSUM banks,
    VectorE drains both banks (+b_out) into one [128, V] tile, one 512 KiB
    contiguous store per row on the SP queue.  The last 8 rows store in
    half-V chunks so the post-matmul tail is short.

The walrus build in this container rejects any instruction carrying more
than one sync wait ("Too many sync wait commands").  fixup_sync_waits()
post-processes the finished module: for every instruction with n>1 waits it
hoists n-1 of them onto fresh single-wait nops on the same engine placed
immediately before it, which is semantically identical on in-order engine
streams.
"""

import numpy as np

import bass_rust
import concourse.bass as bass
import concourse.mybir as mybir
import concourse.tile as tile

B, T, U = 4, 512, 128
D, J, V = 512, 512, 1024
N_CORES = 8
TS = T // 2  # 256 t-rows per core
F32 = mybir.dt.float32
BF16 = mybir.dt.bfloat16


def fixup_sync_waits(nc: bass.Bass) -> None:
    n_split = 0
    for fn in nc.m.functions:
        for bb in fn.blocks:
            insts = bb.instructions
            if not any(
                i.sync_info is not None and len(i.sync_info.on_wait) > 1
                for i in insts
            ):
                continue
            new = []
            for i in insts:
                si = i.sync_info
                if si is not None and len(si.on_wait) > 1:
                    waits = list(si.on_wait)
                    for w in waits[:-1]:
                        nop = mybir.InstNoOp(
                            name=f"{i.name}-wsplit-{n_split}", ins=[], outs=[]
                        )
                        n_split += 1
                        nop.engine = i.engine
                        nop.sync_info = bass_rust.SyncInfo(
                            on_wait=[w], on_update=[]
                        )
                        new.append(nop)
                    i.sync_info = bass_rust.SyncInfo(
                        on_wait=[waits[-1]], on_update=list(si.on_update)
                    )
                new.append(i)
            bb.instructions = new


def build_kernel() -> bass.Bass:
    nc = bass.Bass()
    enc = nc.declare_dram_parameter("enc", [TS, D], BF16, isOutput=False)
    dec = nc.declare_dram_parameter("dec", [U, D], BF16, isOutput=False)
    w_enc = nc.declare_dram_parameter("w_enc", [D, J], BF16, isOutput=False)
    w_dec = nc.declare_dram_parameter("w_dec", [D, J], BF16, isOutput=False)
    w_out = nc.declare_dram_parameter("w_out", [J, V], BF16, isOutput=False)
    b_sum = nc.declare_dram_parameter("b_sum", [J], F32, isOutput=False)
    b_out = nc.declare_dram_parameter("b_out", [V], F32, isOutput=False)
    out = nc.declare_dram_parameter("out", [TS, U, V], F32, isOutput=True)

    JC = J // 128  # 4 contraction chunks of the joint dim
    DC = D // 128  # 4 chunks of the input-feature dim
    MC = TS // 128  # 2 chunks of this core's t rows
    VB = V // 512  # 2 PSUM banks of the vocab dim
    Tanh = mybir.ActivationFunctionType.Tanh

    with tile.TileContext(nc) as tc:
        with (
            tc.tile_pool(name="const", bufs=1) as const,
            tc.tile_pool(name="stage", bufs=2) as stage,
            tc.tile_pool(name="joint", bufs=4) as jpool,
            tc.tile_pool(name="osb", bufs=4) as opool,
            tc.tile_pool(name="ps_tr", bufs=2, space="PSUM") as ps_tr,
            tc.tile_pool(name="ps_pre", bufs=2, space="PSUM") as ps_pre,
            tc.tile_pool(name="ps_main", bufs=4, space="PSUM") as ps_main,
        ):
            from concourse.masks import make_identity

            ident = const.tile([128, 128], BF16)
            make_identity(nc, ident[:])

            # ---- loads; HWDGE zip-interleaves SP/Act so the serialized
            # transfer order is:
            # enc, w_enc, dec, w_dec, b_sum, w_out[v0], w_out[v1], b_out
            enc_sb = stage.tile([128, MC, D], BF16)
            nc.sync.dma_start(
                out=enc_sb[:], in_=enc.rearrange("(mo mi) d -> mi mo d", mi=128)
            )
            wenc_sb = const.tile([128, DC, J], BF16)
            nc.scalar.dma_start(
                out=wenc_sb[:], in_=w_enc.rearrange("(po pi) f -> pi po f", pi=128)
            )
            dec_sb = stage.tile([128, D], BF16)
            nc.sync.dma_start(out=dec_sb[:], in_=dec[:])
            wdec_sb = const.tile([128, DC, J], BF16)
            nc.scalar.dma_start(
                out=wdec_sb[:], in_=w_dec.rearrange("(po pi) f -> pi po f", pi=128)
            )
            bsum_sb = const.tile([128, JC], F32)
            nc.sync.dma_start(
                out=bsum_sb[:], in_=b_sum.rearrange("(o p) -> p o", p=128)
            )
            wvr = w_out.rearrange("(po pi) f -> pi po f", pi=128)
            wout_v = []
            for vc in range(VB):
                wv = const.tile([128, JC, 512], BF16, tag=f"wout{vc}")
                eng = nc.scalar if vc == 0 else nc.sync
                eng.dma_start(out=wv[:], in_=wvr[:, :, vc * 512 : (vc + 1) * 512])
                wout_v.append(wv)
            bout_bc = const.tile([128, V], F32)
            nc.scalar.dma_start(out=bout_bc[:], in_=b_out[:].partition_broadcast(128))

            # ---- preload the Tanh activation table while loads are in
            # flight, so the first real activation doesn't pay the load
            actwarm = stage.tile([128, 1], BF16, tag="actwarm")
            nc.scalar.activation(actwarm[:], ident[:, 0:1], Tanh, scale=1.0)

            # ---- PE p-state warmup: keep the PE busy on dummy matmuls while
            # the first loads are in flight, so the clock is fully ramped
            # (2.4 GHz needs ~3 us of continuous busy) when real work starts
            warm = ps_pre.tile([128, 512], F32, tag="pre")
            for _ in range(28):
                nc.tensor.matmul(warm[:, :128], lhsT=ident[:], rhs=ident[:])

            # ---- transpose enc (TS x D) -> encT [d_inner, dc*m] ----
            # all 8 transposes land in ONE full-bank PSUM tile (256 B each),
            # drained by a single wide DVE copy: the PE never waits on DVE
            trb = ps_tr.tile([128, DC, MC, 128], BF16, tag="trb")
            for mc in range(MC):
                for dc in range(DC):
                    nc.tensor.transpose(
                        trb[:, dc, mc], enc_sb[:, mc, dc * 128 : (dc + 1) * 128],
                        ident[:],
                    )
            encT = const.tile([128, DC, TS], BF16)
            nc.vector.tensor_copy(encT[:], trb[:])

            # ---- transpose dec (U x D) -> decT [d_inner, dc, u] ----
            trd = ps_tr.tile([128, DC, MC, 128], BF16, tag="trb")
            for dc in range(DC):
                nc.tensor.transpose(
                    trd[:, dc, 0], dec_sb[:, dc * 128 : (dc + 1) * 128], ident[:]
                )
            decT = const.tile([128, DC, U], BF16)
            nc.vector.tensor_copy(decT[:], trd[:, :, 0])

            # ---- enc_projT[j, m] (+ bsum); PSUM from the 4-deep mm pool so
            # chunk drains never gate the next chunk's matmuls ----
            encb = const.tile([128, JC, TS], F32)
            for jc in range(JC):
                pe = ps_main.tile([128, 512], F32, tag="mm")
                for dc in range(DC):
                    nc.tensor.matmul(
                        pe[:, :TS],
                        lhsT=wenc_sb[:, dc, jc * 128 : (jc + 1) * 128],
                        rhs=encT[:, dc],
                        start=(dc == 0),
                        stop=(dc == DC - 1),
                    )
                nc.vector.tensor_scalar(
                    encb[:, jc], pe[:, :TS], bsum_sb[:, jc : jc + 1], None,
                    mybir.AluOpType.add,
                )

            # ---- dec_projT[j, u] directly (16 narrow matmuls at full
            # p-state); all 4 jc-groups land in one PSUM bank, drained
            # per-chunk straight into the bf16 decp tiles ----
            dpj = ps_pre.tile([128, JC, U], F32, tag="pre")
            for jc in range(JC):
                for dc in range(DC):
                    nc.tensor.matmul(
                        dpj[:, jc],
                        lhsT=wdec_sb[:, dc, jc * 128 : (jc + 1) * 128],
                        rhs=decT[:, dc],
                        start=(dc == 0),
                        stop=(dc == DC - 1),
                    )
            decp = []
            for jc in range(JC):
                dtile = const.tile([128, U], BF16, tag=f"decp{jc}")
                nc.vector.tensor_copy(dtile[:], dpj[:, jc])
                decp.append(dtile)

            # ---- main loop over this core's 256 t rows ----
            for t in range(TS):
                jts = []
                for jc in range(JC):
                    jt = jpool.tile([128, U], BF16, tag=f"jt{jc}")
                    nc.scalar.activation(
                        jt[:],
                        decp[jc][:],
                        Tanh,
                        bias=encb[:, jc, t : t + 1],
                        scale=1.0,
                    )
                    jts.append(jt)
                taper = t >= TS - 8
                osb = None if taper else opool.tile([128, V], F32, tag="osb")
                for vc in range(VB):
                    po = ps_main.tile([128, 512], F32, tag="mm")
                    for jc in range(JC):
                        nc.tensor.matmul(
                            po[:],
                            lhsT=jts[jc][:],
                            rhs=wout_v[vc][:, jc],
                            start=(jc == 0),
                            stop=(jc == JC - 1),
                        )
                    if taper:
                        # split the final rows into half-V stores so the tail
                        # after the last matmul is one short drain + DMA
                        oh = opool.tile([128, 512], F32, tag=f"osbl{vc}")
                        nc.vector.tensor_tensor(
                            oh[:], po[:], bout_bc[:, vc * 512 : (vc + 1) * 512],
                            mybir.AluOpType.add,
                        )
                        nc.sync.dma_start(
                            out=out[t, :, vc * 512 : (vc + 1) * 512], in_=oh[:]
                        )
                    else:
                        nc.vector.tensor_tensor(
                            osb[:, vc * 512 : (vc + 1) * 512],
                            po[:],
                            bout_bc[:, vc * 512 : (vc + 1) * 512],
                            mybir.AluOpType.add,
                        )
                if not taper:
                    nc.sync.dma_start(out=out[t], in_=osb[:])

    fixup_sync_waits(nc)
    return nc


_NC_CACHE = None


def _get_nc():
    global _NC_CACHE
    if _NC_CACHE is None:
        _NC_CACHE = build_kernel()
    return _NC_CACHE


def shard_inputs(
    enc_out, dec_out, W_enc, b_enc, W_dec, b_dec, W_out, b_out
) -> list[dict]:
    bf16 = mybir.dt.np(BF16)
    enc_out = np.asarray(enc_out, dtype=np.float32).astype(bf16)
    dec_out = np.ascontiguousarray(np.asarray(dec_out, dtype=np.float32).astype(bf16))
    shared = {
        "w_enc": np.ascontiguousarray(np.asarray(W_enc, dtype=np.float32).astype(bf16)),
        "w_dec": np.ascontiguousarray(np.asarray(W_dec, dtype=np.float32).astype(bf16)),
        "w_out": np.ascontiguousarray(np.asarray(W_out, dtype=np.float32).astype(bf16)),
        "b_sum": np.ascontiguousarray(
            np.asarray(b_enc, dtype=np.float32) + np.asarray(b_dec, dtype=np.float32)
        ),
        "b_out": np.ascontiguousarray(np.asarray(b_out, dtype=np.float32)),
    }
    in_maps = []
    for c in range(N_CORES):
        b, t0 = c // 2, (c % 2) * TS
        in_maps.append(
            {
                "enc": np.ascontiguousarray(enc_out[b, t0 : t0 + TS, 0, :]),
                "dec": np.ascontiguousarray(dec_out[b, 0, :, :]),
                **shared,
            }
        )
    return in_maps


def unshard_output(results: list[dict]) -> np.ndarray:
    out = np.empty((B, T, U, V), dtype=np.float32)
    for c, r in enumerate(results):
        b, t0 = c // 2, (c % 2) * TS
        out[b, t0 : t0 + TS] = r["out"]
    return out


def run_sharded(in_maps, **kwargs):
    from concourse.bass_utils import run_bass_kernel_spmd

    return run_bass_kernel_spmd(_get_nc(), in_maps, list(range(N_CORES)), **kwargs)


def kernel(enc_out, dec_out, W_enc, b_enc, W_dec, b_dec, W_out, b_out) -> np.ndarray:
    in_maps = shard_inputs(enc_out, dec_out, W_enc, b_enc, W_dec, b_dec, W_out, b_out)
    res = run_sharded(in_maps)
    return unshard_output(res.results)
